# revision 2
# baseline (speedup 1.0000x reference)
"""AutoRegressiveGraphConvLayer TRN2 Bass kernel.

Math (reference has NO activation between the two linears of each aggregation
MLP, so each MLP folds into a single linear):
  pair_n[k] = relu(nodes[src]@An + nodes[dst]@Bn + edges[k]@Cn + bn)
  pair_e[k] = relu(nodes[src]@Ae + edges[k]@Ce + be)
  agg_n[i]  = mean_t pair_n  (full window)
  mean_e[k] = exclusive windowed prefix-mean of pair_e
  out_edges = relu(mean_e@Wet + edges@Web + bias_e)
  out_nodes = relu(agg_n@Wna + nodes@Wnb + bias_e)   (reference bug: bias_e)

Layout strategy: everything on-chip is [feature(partition), edge(col)] with
bf16 operands and fp32 PSUM/state. Window ops (prefix means, window sums) are
DVE tensor_tensor_scans with a repeating reset mask. src/dst gathers are
matmul moving-operand access patterns (overlapping / step-0 col APs) over the
node table. Outputs are produced transposed ([64, E]) and the host transposes
back. Sharding: batch (8) across the 8 cores; weights/masks replicated.
"""
import sys
import os

sys.path.insert(0, "/opt/trn_rl_repo")

import numpy as np
import ml_dtypes
from contextlib import ExitStack

import concourse.bacc as bacc
import concourse.mybir as mybir
import concourse.tile as tile
from concourse.bass_utils import run_bass_kernel_spmd

bf16 = ml_dtypes.bfloat16
F32 = mybir.dt.float32
BF = mybir.dt.bfloat16

N, M, B = 1024, 64, 8
FD = 64
NHEAD = 64                  # nodes 0..63 have triangular windows
E_HEAD = NHEAD * M          # padded head edges (4096)
E_BODY = (N - NHEAD) * M    # 61440
E_REAL = 63456
MACRO = 1024                # edges per macro-tile (16 windows)
N_BODY_T = E_BODY // MACRO  # 60
N_HEAD_T = E_HEAD // MACRO  # 4

# const blob column layout (bf16, 128 partitions)
_CB_SLOTS = dict(
    wc=128,       # [Cn|Ce] at rows 64:128 (matches rhs base partition 64)
    wsrc=128,     # [An|Ae] rows 0:64
    wdst=64,      # Bn rows 0:64
    wout=64,      # [Wet;Web] rows 0:128
    wnode=64,     # [Wna;Wnb] rows 0:128
    mask=1024,    # reset mask: rows 0:64 AND rows 64:128 (same content)
    stile=1024,   # s(t)=1/max(t,1), s(0)=0, rows 0:64
    invw=1024,    # 1/max(w_i,1) per node col, rows 0:64
    tri=4096,     # head validity mask (node,t): t<i, rows 0:64
)
CBW = sum(_CB_SLOTS.values())


def _cb_off(name):
    o = 0
    for k, v in _CB_SLOTS.items():
        if k == name:
            return o
        o += v
    raise KeyError(name)


def _indices():
    w = np.minimum(np.arange(N), M)
    node_start = np.concatenate([[0], np.cumsum(w)[:-1]]).astype(np.int64)
    return w, node_start


def _build_module(reps=1):
    nc = bacc.Bacc("TRN2", target_bir_lowering=False, debug=False)
    d_eb = nc.dram_tensor("eb", [FD, E_BODY], BF, kind="ExternalInput")
    d_eh = nc.dram_tensor("eh", [FD, E_HEAD], BF, kind="ExternalInput")
    d_nt = nc.dram_tensor("nt", [FD, N], BF, kind="ExternalInput")
    d_cb = nc.dram_tensor("cblob", [128, CBW], BF, kind="ExternalInput")
    d_fb = nc.dram_tensor("fblob", [128, 2], F32, kind="ExternalInput")
    d_ob = nc.dram_tensor("ob", [FD, E_BODY], BF, kind="ExternalOutput")
    d_oh = nc.dram_tensor("oh", [FD, E_HEAD], BF, kind="ExternalOutput")
    d_on = nc.dram_tensor("onod", [FD, N], F32, kind="ExternalOutput")

    with tile.TileContext(nc) as tc, ExitStack() as ctx:
        const = ctx.enter_context(tc.tile_pool(name="const", bufs=1))
        work = ctx.enter_context(tc.tile_pool(name="work", bufs=3))
        owork = ctx.enter_context(tc.tile_pool(name="owork", bufs=2))
        pp_pool = ctx.enter_context(tc.tile_pool(name="pp", bufs=2, space="PSUM"))
        po_pool = ctx.enter_context(tc.tile_pool(name="po", bufs=2, space="PSUM"))

        cb = const.tile([128, CBW], BF)
        nc.sync.dma_start(cb[:], d_cb[:])
        fb = const.tile([128, 2], F32)
        nc.sync.dma_start(fb[:], d_fb[:])
        nodesT = const.tile([FD, N], BF)
        nc.sync.dma_start(nodesT[:], d_nt[:])
        catN = const.tile([128, N], BF)   # rows 0:64 agg*invw, rows 64:128 nodesT
        nc.sync.dma_start(catN[FD:128, :], d_nt[:])

        o = _cb_off
        wc = cb[64:128, o("wc"):o("wc") + 128]
        wsrc = cb[0:FD, o("wsrc"):o("wsrc") + 128]
        wdst = cb[0:FD, o("wdst"):o("wdst") + FD]
        wout = cb[:, o("wout"):o("wout") + FD]
        wnode = cb[:, o("wnode"):o("wnode") + FD]
        mask_lo = cb[0:FD, o("mask"):o("mask") + MACRO]
        mask_hi = cb[FD:128, o("mask"):o("mask") + MACRO]
        stile = cb[0:FD, o("stile"):o("stile") + MACRO]
        invw = cb[0:FD, o("invw"):o("invw") + N]

        # pair ring: [128, 1+MACRO], col 0 is the scan shift-in pad (zeroed on
        # ACT so the relu's pad dep is same-engine; also warms ACT past fb).
        pair_ring = []
        for k in range(3):
            p = const.tile([128, 1 + MACRO], BF, tag=f"pair{k}")
            nc.scalar.mul(p[:, 0:1], fb[:, 0:1], 0.0)
            pair_ring.append(p)
        # clock-warm DVE/GPS past the const-blob DMA
        wv = const.tile([128, 1], BF)
        nc.vector.tensor_copy(wv[:], cb[:, 0:1])
        wg = const.tile([128, 1], BF)
        nc.gpsimd.tensor_copy(wg[:], cb[:, 1:2])

        def rhs_ap(base_ap, dims, offset_cols):
            ap = base_ap.copy()
            ap.ap = mybir.VecI64Pair(dims)
            ap.offset = ap.offset + offset_cols
            return ap

        loop_ctx = tc.For_i(0, reps, 1) if reps > 1 else None
        if loop_ctx is not None:
            ctx.enter_context(loop_ctx)
        n_tiles = N_HEAD_T + N_BODY_T
        po = None
        for mt in range(n_tiles):
            head = mt < N_HEAD_T
            if head:
                node0 = 16 * mt
                src_dram = d_eh
                ecol0 = MACRO * mt
            else:
                bmt = mt - N_HEAD_T
                node0 = NHEAD + 16 * bmt
                src_dram = d_eb
                ecol0 = MACRO * bmt

            catT = work.tile([128, MACRO], BF, tag="catT")
            nc.sync.dma_start(catT[FD:128, :], src_dram[:, ecol0:ecol0 + MACRO])

            pp = pp_pool.tile([128, MACRO], F32, tag="pp")
            nodesT_ap = nodesT[:].copy()
            for h in range(2):
                c0 = 512 * h
                n0 = node0 + 8 * h
                nc.tensor.matmul(pp[:, c0:c0 + 512], wc,
                                 catT[FD:128, c0:c0 + 512],
                                 start=True, stop=False)
                dst_ap = rhs_ap(nodesT_ap, [[N, FD], [1, 8], [0, M]], n0)
                nc.tensor.matmul(pp[0:FD, c0:c0 + 512], wdst, dst_ap,
                                 start=False, stop=False)
                if head:
                    s_ap = rhs_ap(nodesT_ap, [[N, FD], [0, 8], [1, M]], 0)
                else:
                    s_ap = rhs_ap(nodesT_ap, [[N, FD], [1, 8], [1, M]], n0 - M)
                nc.tensor.matmul(pp[:, c0:c0 + 512], wsrc, s_ap,
                                 start=False, stop=True)

            pair = pair_ring[mt % 3]
            nc.scalar.activation(pair[:, 1:1 + MACRO], pp[:],
                                 mybir.ActivationFunctionType.Relu,
                                 bias=fb[:, 0:1])
            if head:
                tri = cb[0:FD, o("tri") + ecol0:o("tri") + ecol0 + MACRO]
                nc.gpsimd.tensor_tensor(pair[0:FD, 1:1 + MACRO],
                                        pair[0:FD, 1:1 + MACRO], tri,
                                        mybir.AluOpType.mult)

            # exclusive windowed prefix of pair_e: state=(state+pair(t-1))*m(t)
            excl = work.tile([FD, MACRO], BF, tag="excl")
            nc.vector.tensor_tensor_scan(excl[:], pair[FD:128, 0:MACRO], mask_hi,
                                         0.0, mybir.AluOpType.add,
                                         mybir.AluOpType.mult)
            # mean_e = excl * s  -> catT mean half (rows 0:64)
            nc.vector.tensor_tensor(catT[0:FD, :], excl[:], stile,
                                    mybir.AluOpType.mult)
            # inclusive windowed sums of pair_n: state=m(t)*state+pair(t)
            sn = work.tile([FD, MACRO], BF, tag="sn")
            nc.vector.tensor_tensor_scan(sn[:], mask_lo, pair[0:FD, 1:1 + MACRO],
                                         0.0, mybir.AluOpType.mult,
                                         mybir.AluOpType.add)
            # agg*invw -> catN rows 0:64 (sample col 63 of each window)
            sn_s = sn[:].copy()
            sn_s.ap = mybir.VecI64Pair([[MACRO, FD], [M, 16]])
            sn_s.offset = sn_s.offset + (M - 1)
            nc.vector.tensor_tensor(catN[0:FD, node0:node0 + 16], sn_s,
                                    invw.copy()[:, node0:node0 + 16],
                                    mybir.AluOpType.mult)

            # output projection, [o, e] layout, two macro-tiles share a psum
            if mt % 2 == 0:
                po = po_pool.tile([128, MACRO], F32, tag="po")
            r0 = 64 * (mt % 2)
            for h in range(2):
                c0 = 512 * h
                nc.tensor.matmul(po[r0:r0 + FD, c0:c0 + 512], wout,
                                 catT[:, c0:c0 + 512], start=True, stop=True)
            if mt % 2 == 1 or mt == n_tiles - 1:
                outT = owork.tile([128, MACRO], BF, tag="outT")
                nc.scalar.activation(outT[:], po[:],
                                     mybir.ActivationFunctionType.Relu,
                                     bias=fb[:, 1:2])
                for q, mtq in enumerate((mt - (mt % 2), mt - (mt % 2) + 1)):
                    if mtq >= n_tiles:
                        continue
                    if mtq < N_HEAD_T:
                        dd, cc = d_oh, MACRO * mtq
                    else:
                        dd, cc = d_ob, MACRO * (mtq - N_HEAD_T)
                    nc.sync.dma_start(dd[:, cc:cc + MACRO],
                                      outT[64 * q:64 * q + FD, :])

        # node outputs: relu([agg*invw ; nodes] @ [Wna;Wnb] + bias_e)
        pn = po_pool.tile([FD, N], F32, tag="po")
        for h in range(2):
            c0 = 512 * h
            nc.tensor.matmul(pn[:, c0:c0 + 512], wnode, catN[:, c0:c0 + 512],
                             start=True, stop=True)
        outN = owork.tile([FD, N], F32, tag="outN")
        nc.scalar.activation(outN[:], pn[:], mybir.ActivationFunctionType.Relu,
                             bias=fb[0:FD, 1:2])
        nc.sync.dma_start(d_on[:], outN[:])

    nc.compile()
    return nc


def _host_consts(inp):
    f32 = np.float32
    Wn1, Wn2 = inp["Wn1"].astype(f32), inp["Wn2"].astype(f32)
    We1, We2 = inp["We1"].astype(f32), inp["We2"].astype(f32)
    An = Wn1[0:64] @ Wn2
    Bn = Wn1[64:128] @ Wn2
    Cn = Wn1[128:192] @ Wn2
    Ae = We1[0:64] @ We2
    Ce = We1[64:128] @ We2
    bn = inp["bn1"].astype(f32) @ Wn2 + inp["bn2"].astype(f32)
    be = inp["be1"].astype(f32) @ We2 + inp["be2"].astype(f32)
    W_edges = inp["W_edges"].astype(f32)
    W_nodes = inp["W_nodes"].astype(f32)
    bias_e = inp["bias_edges"].astype(f32)

    cblob = np.zeros((128, CBW), bf16)
    o = _cb_off
    cblob[64:128, o("wc"):o("wc") + 128] = np.hstack([Cn, Ce]).astype(bf16)
    cblob[0:64, o("wsrc"):o("wsrc") + 128] = np.hstack([An, Ae]).astype(bf16)
    cblob[0:64, o("wdst"):o("wdst") + FD] = Bn.astype(bf16)
    cblob[:, o("wout"):o("wout") + FD] = W_edges.astype(bf16)
    cblob[:, o("wnode"):o("wnode") + FD] = W_nodes.astype(bf16)
    m_vec = np.ones(M, f32); m_vec[0] = 0.0
    mrow = np.tile(m_vec, MACRO // M)
    cblob[0:64, o("mask"):o("mask") + MACRO] = np.broadcast_to(mrow, (64, MACRO)).astype(bf16)
    cblob[64:128, o("mask"):o("mask") + MACRO] = np.broadcast_to(mrow, (64, MACRO)).astype(bf16)
    s_vec = 1.0 / np.maximum(np.arange(M), 1).astype(f32); s_vec[0] = 0.0
    srow = np.tile(s_vec, MACRO // M)
    cblob[0:64, o("stile"):o("stile") + MACRO] = np.broadcast_to(srow, (64, MACRO)).astype(bf16)
    w, _ = _indices()
    invw_row = (1.0 / np.maximum(w, 1)).astype(f32)
    cblob[0:64, o("invw"):o("invw") + N] = np.broadcast_to(invw_row, (64, N)).astype(bf16)
    t = np.arange(M)
    tri = (t[None, :] < np.arange(NHEAD)[:, None]).astype(f32)  # [node, t]
    trirow = tri.reshape(-1)
    cblob[0:64, o("tri"):o("tri") + E_HEAD] = np.broadcast_to(trirow, (64, E_HEAD)).astype(bf16)

    fblob = np.zeros((128, 2), np.float32)
    fblob[:, 0] = np.concatenate([bn, be])
    fblob[:, 1] = np.concatenate([bias_e, bias_e])
    return cblob, fblob


_CACHE = {}


def _get_nc():
    if "nc" not in _CACHE:
        _CACHE["nc"] = _build_module()
    return _CACHE["nc"]


def kernel(**inputs):
    nc = _get_nc()
    cblob, fblob = _host_consts(inputs)
    w, node_start = _indices()

    nodes = np.asarray(inputs["input_nodes"], np.float32)
    edges = np.asarray(inputs["input_edges"], np.float32)

    # head padding: (node i, t) -> edge node_start[i]+t if t < i else 0
    t = np.arange(M)
    k_pad = node_start[:NHEAD, None] + t[None, :]
    valid = t[None, :] < w[:NHEAD, None]
    k_pad = np.where(valid, k_pad, 0)

    in_maps = []
    for b in range(B):
        eh = edges[b][k_pad.reshape(-1)] * valid.reshape(-1, 1)
        in_maps.append(dict(
            eb=np.ascontiguousarray(edges[b, 2016:].T.astype(bf16)),
            eh=np.ascontiguousarray(eh.T.astype(bf16)),
            nt=np.ascontiguousarray(nodes[b].T.astype(bf16)),
            cblob=cblob, fblob=fblob,
        ))

    res = run_bass_kernel_spmd(nc, in_maps, core_ids=list(range(B)))

    out_nodes = np.empty((B, N, FD), np.float32)
    out_edges = np.empty((B, E_REAL, FD), np.float32)
    vmask = valid.reshape(-1)
    for b in range(B):
        r = res.results[b]
        out_nodes[b] = r["onod"].T
        out_edges[b, :2016] = r["oh"].T.astype(np.float32)[vmask]
        out_edges[b, 2016:] = r["ob"].T.astype(np.float32)
    return out_nodes, out_edges


# revision 3
# speedup vs baseline: 1.5561x; 1.5561x over previous
"""AutoRegressiveGraphConvLayer TRN2 Bass kernel.

Math (reference has NO activation between the two linears of each aggregation
MLP, so each MLP folds into a single linear):
  pair_n[k] = relu(nodes[src]@An + nodes[dst]@Bn + edges[k]@Cn + bn)
  pair_e[k] = relu(nodes[src]@Ae + edges[k]@Ce + be)
  agg_n[i]  = mean_t pair_n  (full window)
  mean_e[k] = exclusive windowed prefix-mean of pair_e
  out_edges = relu(mean_e@Wet + edges@Web + bias_e)
  out_nodes = relu(agg_n@Wna + nodes@Wnb + bias_e)   (reference bug: bias_e)

Layout strategy: everything on-chip is [feature(partition), edge(col)] with
bf16 operands and fp32 PSUM/state. Window ops (prefix means, window sums) are
DVE tensor_tensor_scans with a repeating reset mask. src/dst gathers are
matmul moving-operand access patterns (overlapping / step-0 col APs) over the
node table. Outputs are produced transposed ([64, E]) and the host transposes
back. Sharding: batch (8) across the 8 cores; weights/masks replicated.
"""
import sys
import os

sys.path.insert(0, "/opt/trn_rl_repo")

import numpy as np
import ml_dtypes
from contextlib import ExitStack

import concourse.bacc as bacc
import concourse.mybir as mybir
import concourse.tile as tile
from concourse.bass_utils import run_bass_kernel_spmd

bf16 = ml_dtypes.bfloat16
F32 = mybir.dt.float32
BF = mybir.dt.bfloat16

N, M, B = 1024, 64, 8
FD = 64
NHEAD = 64                  # nodes 0..63 have triangular windows
E_HEAD = NHEAD * M          # padded head edges (4096)
E_BODY = (N - NHEAD) * M    # 61440
E_REAL = 63456
MACRO = 1024                # edges per macro-tile (16 windows)
N_BODY_T = E_BODY // MACRO  # 60
N_HEAD_T = E_HEAD // MACRO  # 4

# const blob column layout (bf16, 128 partitions)
_CB_SLOTS = dict(
    wc=128,       # [Cn|Ce] at rows 64:128 (matches rhs base partition 64)
    wsrc=128,     # [An|Ae] rows 0:64
    wdst=64,      # Bn rows 0:64
    wout=64,      # [Wet;Web] rows 0:128
    wnode=64,     # [Wna;Wnb] rows 0:128
    mask=1024,    # reset mask: rows 0:64 AND rows 64:128 (same content)
    stile=1024,   # s(t)=1/max(t,1), s(0)=0, rows 0:64
    invw=1024,    # 1/max(w_i,1) per node col, rows 0:64
    tri=4096,     # head validity mask (node,t): t<i, rows 0:64
)
CBW = sum(_CB_SLOTS.values())


def _cb_off(name):
    o = 0
    for k, v in _CB_SLOTS.items():
        if k == name:
            return o
        o += v
    raise KeyError(name)


def _indices():
    w = np.minimum(np.arange(N), M)
    node_start = np.concatenate([[0], np.cumsum(w)[:-1]]).astype(np.int64)
    return w, node_start


def _build_module(reps=1):
    nc = bacc.Bacc("TRN2", target_bir_lowering=False, debug=False)
    d_eb = nc.dram_tensor("eb", [FD, E_BODY], BF, kind="ExternalInput")
    d_eh = nc.dram_tensor("eh", [FD, E_HEAD], BF, kind="ExternalInput")
    d_nt = nc.dram_tensor("nt", [FD, N], BF, kind="ExternalInput")
    d_cb = nc.dram_tensor("cblob", [128, CBW], BF, kind="ExternalInput")
    d_fb = nc.dram_tensor("fblob", [128, 2], F32, kind="ExternalInput")
    d_ob = nc.dram_tensor("ob", [FD, E_BODY], BF, kind="ExternalOutput")
    d_oh = nc.dram_tensor("oh", [FD, E_HEAD], BF, kind="ExternalOutput")
    d_on = nc.dram_tensor("onod", [FD, N], F32, kind="ExternalOutput")

    with tile.TileContext(nc) as tc, ExitStack() as ctx:
        const = ctx.enter_context(tc.tile_pool(name="const", bufs=1))
        work = ctx.enter_context(tc.tile_pool(name="work", bufs=3))
        owork = ctx.enter_context(tc.tile_pool(name="owork", bufs=2))
        pp_pool = ctx.enter_context(tc.tile_pool(name="pp", bufs=2, space="PSUM"))
        po_pool = ctx.enter_context(tc.tile_pool(name="po", bufs=2, space="PSUM"))

        cb = const.tile([128, CBW], BF)
        nc.sync.dma_start(cb[:], d_cb[:])
        fb = const.tile([128, 2], F32)
        nc.sync.dma_start(fb[:], d_fb[:])
        nodesT = const.tile([FD, N], BF)
        nc.sync.dma_start(nodesT[:], d_nt[:])
        catN = const.tile([128, N], BF)   # rows 0:64 agg*invw, rows 64:128 nodesT
        nc.sync.dma_start(catN[FD:128, :], d_nt[:])

        o = _cb_off
        wc = cb[64:128, o("wc"):o("wc") + 128]
        wsrc = cb[0:FD, o("wsrc"):o("wsrc") + 128]
        wdst = cb[0:FD, o("wdst"):o("wdst") + FD]
        wout = cb[:, o("wout"):o("wout") + FD]
        wnode = cb[:, o("wnode"):o("wnode") + FD]
        mask_lo = cb[0:FD, o("mask"):o("mask") + MACRO]
        mask_hi = cb[FD:128, o("mask"):o("mask") + MACRO]
        stile = cb[FD:128, o("stile"):o("stile") + MACRO]
        invw = cb[0:FD, o("invw"):o("invw") + N]

        # pair ring: [128, 1+MACRO], col 0 is the scan shift-in pad (zeroed on
        # ACT so the relu's pad dep is same-engine; also warms ACT past fb).
        pair_ring = []
        for k in range(3):
            p = const.tile([128, 1 + MACRO], BF, tag=f"pair{k}")
            nc.scalar.mul(p[:, 0:1], fb[:, 0:1], 0.0)
            pair_ring.append(p)
        # clock-warm DVE/GPS past the const-blob DMA
        wv = const.tile([128, 1], BF)
        nc.vector.tensor_copy(wv[:], cb[:, 0:1])
        wg = const.tile([128, 1], BF)
        nc.gpsimd.tensor_copy(wg[:], cb[:, 1:2])

        def rhs_ap(base_ap, dims, offset_cols):
            ap = base_ap.copy()
            ap.ap = mybir.VecI64Pair(dims)
            ap.offset = ap.offset + offset_cols
            return ap

        loop_ctx = tc.For_i(0, reps, 1) if reps > 1 else None
        if loop_ctx is not None:
            ctx.enter_context(loop_ctx)
        n_tiles = N_HEAD_T + N_BODY_T
        po = None
        for mt in range(n_tiles):
            head = mt < N_HEAD_T
            if head:
                node0 = 16 * mt
                src_dram = d_eh
                ecol0 = MACRO * mt
            else:
                bmt = mt - N_HEAD_T
                node0 = NHEAD + 16 * bmt
                src_dram = d_eb
                ecol0 = MACRO * bmt

            catT = work.tile([128, MACRO], BF, tag="catT")
            nc.sync.dma_start(catT[FD:128, 0:512], src_dram[:, ecol0:ecol0 + 512])
            nc.sync.dma_start(catT[FD:128, 512:MACRO],
                              src_dram[:, ecol0 + 512:ecol0 + MACRO])

            pp = pp_pool.tile([128, MACRO], F32, tag="pp")
            nodesT_ap = nodesT[:].copy()
            for h in range(2):
                c0 = 512 * h
                nc.tensor.matmul(pp[:, c0:c0 + 512], wc,
                                 catT[FD:128, c0:c0 + 512],
                                 start=True, stop=False)
            for h in range(2):
                c0 = 512 * h
                n0 = node0 + 8 * h
                dst_ap = rhs_ap(nodesT_ap, [[N, FD], [1, 8], [0, M]], n0)
                nc.tensor.matmul(pp[0:FD, c0:c0 + 512], wdst, dst_ap,
                                 start=False, stop=False)
            for h in range(2):
                c0 = 512 * h
                n0 = node0 + 8 * h
                if head:
                    s_ap = rhs_ap(nodesT_ap, [[N, FD], [0, 8], [1, M]], 0)
                else:
                    s_ap = rhs_ap(nodesT_ap, [[N, FD], [1, 8], [1, M]], n0 - M)
                nc.tensor.matmul(pp[:, c0:c0 + 512], wsrc, s_ap,
                                 start=False, stop=True)

            pair = pair_ring[mt % 3]
            nc.scalar.activation(pair[:, 1:1 + MACRO], pp[:],
                                 mybir.ActivationFunctionType.Relu,
                                 bias=fb[:, 0:1])
            if head:
                tri = cb[0:FD, o("tri") + ecol0:o("tri") + ecol0 + MACRO]
                nc.gpsimd.tensor_tensor(pair[0:FD, 1:1 + MACRO],
                                        pair[0:FD, 1:1 + MACRO], tri,
                                        mybir.AluOpType.mult)

            # fused exclusive windowed prefix of BOTH halves:
            # state = (state + pair(t-1)) * m(t); rows 0:64 pair_n, 64:128 pair_e
            excl = work.tile([128, MACRO], BF, tag="excl")
            mask_full = cb[:, o("mask"):o("mask") + MACRO]
            nc.vector.tensor_tensor_scan(excl[:], pair[:, 0:MACRO], mask_full,
                                         0.0, mybir.AluOpType.add,
                                         mybir.AluOpType.mult)
            # mean_e = excl_e * s  -> catT mean half (rows 0:64)
            nc.vector.tensor_tensor(catT[0:FD, :], excl[FD:128, :], stile,
                                    mybir.AluOpType.mult)
            # agg*invw -> catN rows 0:64; window sum = excl_n[63] + pair_n[63]
            ex_s = excl[:].copy()
            ex_s.ap = mybir.VecI64Pair([[MACRO, FD], [M, 16]])
            ex_s.offset = ex_s.offset + (M - 1)
            pr_s = pair[:].copy()
            pr_s.ap = mybir.VecI64Pair([[1 + MACRO, FD], [M, 16]])
            pr_s.offset = pr_s.offset + M  # col 1 + 63
            agg16 = work.tile([FD, 16], BF, tag="agg16")
            nc.gpsimd.tensor_tensor(agg16[:], ex_s, pr_s, mybir.AluOpType.add)
            nc.gpsimd.tensor_tensor(catN[0:FD, node0:node0 + 16], agg16[:],
                                    invw.copy()[:, node0:node0 + 16],
                                    mybir.AluOpType.mult)

            # output projection, [o, e] layout, two macro-tiles share a psum
            if mt % 2 == 0:
                po = po_pool.tile([128, MACRO], F32, tag="po")
            r0 = 64 * (mt % 2)
            for h in range(2):
                c0 = 512 * h
                nc.tensor.matmul(po[r0:r0 + FD, c0:c0 + 512], wout,
                                 catT[:, c0:c0 + 512], start=True, stop=True)
            if mt % 2 == 1 or mt == n_tiles - 1:
                outT = owork.tile([128, MACRO], BF, tag="outT")
                nc.scalar.activation(outT[:], po[:],
                                     mybir.ActivationFunctionType.Relu,
                                     bias=fb[:, 1:2])
                for q, mtq in enumerate((mt - (mt % 2), mt - (mt % 2) + 1)):
                    if mtq >= n_tiles:
                        continue
                    if mtq < N_HEAD_T:
                        dd, cc = d_oh, MACRO * mtq
                    else:
                        dd, cc = d_ob, MACRO * (mtq - N_HEAD_T)
                    nc.sync.dma_start(dd[:, cc:cc + MACRO],
                                      outT[64 * q:64 * q + FD, :])

        # node outputs: relu([agg*invw ; nodes] @ [Wna;Wnb] + bias_e)
        pn = po_pool.tile([FD, N], F32, tag="po")
        for h in range(2):
            c0 = 512 * h
            nc.tensor.matmul(pn[:, c0:c0 + 512], wnode, catN[:, c0:c0 + 512],
                             start=True, stop=True)
        outN = owork.tile([FD, N], F32, tag="outN")
        nc.scalar.activation(outN[:], pn[:], mybir.ActivationFunctionType.Relu,
                             bias=fb[0:FD, 1:2])
        nc.sync.dma_start(d_on[:], outN[:])

    nc.compile()
    return nc


def _host_consts(inp):
    f32 = np.float32
    Wn1, Wn2 = inp["Wn1"].astype(f32), inp["Wn2"].astype(f32)
    We1, We2 = inp["We1"].astype(f32), inp["We2"].astype(f32)
    An = Wn1[0:64] @ Wn2
    Bn = Wn1[64:128] @ Wn2
    Cn = Wn1[128:192] @ Wn2
    Ae = We1[0:64] @ We2
    Ce = We1[64:128] @ We2
    bn = inp["bn1"].astype(f32) @ Wn2 + inp["bn2"].astype(f32)
    be = inp["be1"].astype(f32) @ We2 + inp["be2"].astype(f32)
    W_edges = inp["W_edges"].astype(f32)
    W_nodes = inp["W_nodes"].astype(f32)
    bias_e = inp["bias_edges"].astype(f32)

    cblob = np.zeros((128, CBW), bf16)
    o = _cb_off
    cblob[64:128, o("wc"):o("wc") + 128] = np.hstack([Cn, Ce]).astype(bf16)
    cblob[0:64, o("wsrc"):o("wsrc") + 128] = np.hstack([An, Ae]).astype(bf16)
    cblob[0:64, o("wdst"):o("wdst") + FD] = Bn.astype(bf16)
    cblob[:, o("wout"):o("wout") + FD] = W_edges.astype(bf16)
    cblob[:, o("wnode"):o("wnode") + FD] = W_nodes.astype(bf16)
    m_vec = np.ones(M, f32); m_vec[0] = 0.0
    mrow = np.tile(m_vec, MACRO // M)
    cblob[0:64, o("mask"):o("mask") + MACRO] = np.broadcast_to(mrow, (64, MACRO)).astype(bf16)
    cblob[64:128, o("mask"):o("mask") + MACRO] = np.broadcast_to(mrow, (64, MACRO)).astype(bf16)
    s_vec = 1.0 / np.maximum(np.arange(M), 1).astype(f32); s_vec[0] = 0.0
    srow = np.tile(s_vec, MACRO // M)
    cblob[64:128, o("stile"):o("stile") + MACRO] = np.broadcast_to(srow, (64, MACRO)).astype(bf16)
    w, _ = _indices()
    invw_row = (1.0 / np.maximum(w, 1)).astype(f32)
    cblob[0:64, o("invw"):o("invw") + N] = np.broadcast_to(invw_row, (64, N)).astype(bf16)
    t = np.arange(M)
    tri = (t[None, :] < np.arange(NHEAD)[:, None]).astype(f32)  # [node, t]
    trirow = tri.reshape(-1)
    cblob[0:64, o("tri"):o("tri") + E_HEAD] = np.broadcast_to(trirow, (64, E_HEAD)).astype(bf16)

    fblob = np.zeros((128, 2), np.float32)
    fblob[:, 0] = np.concatenate([bn, be])
    fblob[:, 1] = np.concatenate([bias_e, bias_e])
    return cblob, fblob


_CACHE = {}


def _get_nc():
    if "nc" not in _CACHE:
        _CACHE["nc"] = _build_module()
    return _CACHE["nc"]


def kernel(**inputs):
    nc = _get_nc()
    cblob, fblob = _host_consts(inputs)
    w, node_start = _indices()

    nodes = np.asarray(inputs["input_nodes"], np.float32)
    edges = np.asarray(inputs["input_edges"], np.float32)

    # head padding: (node i, t) -> edge node_start[i]+t if t < i else 0
    t = np.arange(M)
    k_pad = node_start[:NHEAD, None] + t[None, :]
    valid = t[None, :] < w[:NHEAD, None]
    k_pad = np.where(valid, k_pad, 0)

    in_maps = []
    for b in range(B):
        eh = edges[b][k_pad.reshape(-1)] * valid.reshape(-1, 1)
        in_maps.append(dict(
            eb=np.ascontiguousarray(edges[b, 2016:].T.astype(bf16)),
            eh=np.ascontiguousarray(eh.T.astype(bf16)),
            nt=np.ascontiguousarray(nodes[b].T.astype(bf16)),
            cblob=cblob, fblob=fblob,
        ))

    res = run_bass_kernel_spmd(nc, in_maps, core_ids=list(range(B)))

    out_nodes = np.empty((B, N, FD), np.float32)
    out_edges = np.empty((B, E_REAL, FD), np.float32)
    vmask = valid.reshape(-1)
    for b in range(B):
        r = res.results[b]
        out_nodes[b] = r["onod"].T
        out_edges[b, :2016] = r["oh"].T.astype(np.float32)[vmask]
        out_edges[b, 2016:] = r["ob"].T.astype(np.float32)
    return out_nodes, out_edges
